# revision 3
# baseline (speedup 1.0000x reference)
"""Trainium2 Bass kernel for nn_DGNN (gnn_message_passing).

Reference computation (B=4, N=8192, F=32):
    delay_steps = time_delay // 5
    active      = (t >= delay_steps) & (adj > 0)
    A           = where(active, adj, 0)              # == adj * (time_delay <= 5*t+4)
    adjusted    = einsum('ij,bjf->bif', A, x)
    h           = relu(adjusted @ W1 + b1)
    out         = sigmoid(h @ W2 + b2)

Sharding: destination nodes i are split row-wise across 8 cores (1024 each).
Each core streams its slice of adj^T / time_delay^T ([8192 j, 1024 i], j on
partitions so the PE can contract over j), masks adj with the time-delay
predicate on the DVE, and accumulates adjusted^T in PSUM with the 4 batches
packed side-by-side in the stationary operand (partition q = 32*b + f).
The tiny per-node MLP runs on-device with block-diagonal W1/W2 so all 4
batches share one matmul. Output is returned transposed per core and
unsharded on the host.
"""

import numpy as np

B = 4
N = 8192
F = 32
P = 128
NCORES = 8
NI = N // NCORES  # dest-nodes per core
JT = N // P       # contraction tiles

MM_N = 512        # moving-operand free dim per matmul (fp32 max)


def _build(nj, ni, thr, mm_dtype_name="float32"):
    """Trace + compile the per-core Bass program.

    nj: contraction size (source nodes), ni: dest nodes per core,
    thr: mask threshold (keep edge iff time_delay <= thr).
    """
    from contextlib import ExitStack

    import concourse.bacc as bacc
    import concourse.mybir as mybir
    import concourse.tile as tile

    f32 = mybir.dt.float32
    i32 = mybir.dt.int32
    mm_dt = getattr(mybir.dt, mm_dtype_name)

    jt_n = nj // P
    mm_n = min(MM_N, ni)

    nc = bacc.Bacc("TRN2", target_bir_lowering=False, debug=False)

    adjT_d = nc.dram_tensor("adjT", [nj, ni], f32, kind="ExternalInput").ap()
    tdT_d = nc.dram_tensor("tdT", [nj, ni], i32, kind="ExternalInput").ap()
    xsb_d = nc.dram_tensor("xsb", [P, jt_n * P], f32, kind="ExternalInput").ap()
    bd1_d = nc.dram_tensor("bd1", [P, P], f32, kind="ExternalInput").ap()
    bd2_d = nc.dram_tensor("bd2", [P, P], f32, kind="ExternalInput").ap()
    bias1_d = nc.dram_tensor("bias1", [P, 1], f32, kind="ExternalInput").ap()
    bias2_d = nc.dram_tensor("bias2", [P, 1], f32, kind="ExternalInput").ap()
    outT_d = nc.dram_tensor("outT", [P, ni], f32, kind="ExternalOutput").ap()

    with tile.TileContext(nc) as tc, ExitStack() as ctx:
        io = ctx.enter_context(tc.tile_pool(name="io", bufs=3))
        wrk = ctx.enter_context(tc.tile_pool(name="wrk", bufs=3))
        singles = ctx.enter_context(tc.tile_pool(name="singles", bufs=1))
        pp = ctx.enter_context(tc.tile_pool(name="pp", bufs=1, space="PSUM"))

        x_t = singles.tile([P, jt_n * P], f32)
        nc.sync.dma_start(out=x_t, in_=xsb_d)
        bd1_t = singles.tile([P, P], f32)
        nc.sync.dma_start(out=bd1_t, in_=bd1_d)
        bd2_t = singles.tile([P, P], f32)
        nc.sync.dma_start(out=bd2_t, in_=bd2_d)
        bias1_t = singles.tile([P, 1], f32)
        nc.sync.dma_start(out=bias1_t, in_=bias1_d)
        bias2_t = singles.tile([P, 1], f32)
        nc.sync.dma_start(out=bias2_t, in_=bias2_d)

        psum_main = pp.tile([P, ni], f32)

        for jt in range(jt_n):
            adj_t = io.tile([P, ni], f32, tag="adj")
            nc.sync.dma_start(out=adj_t, in_=adjT_d[jt * P : (jt + 1) * P, :])
            td_t = io.tile([P, ni], i32, tag="td")
            nc.sync.dma_start(out=td_t, in_=tdT_d[jt * P : (jt + 1) * P, :])

            mask_t = wrk.tile([P, ni], f32, tag="mask")
            nc.vector.tensor_scalar(
                mask_t, td_t, float(thr), None, op0=mybir.AluOpType.is_le
            )
            a_t = wrk.tile([P, ni], mm_dt, tag="a")
            nc.vector.tensor_mul(a_t, adj_t, mask_t)

            lhsT = x_t[:, jt * P : (jt + 1) * P]
            if mm_dt != f32:
                lhsT = lhsT.bitcast(mm_dt)
            for h in range(ni // mm_n):
                nc.tensor.matmul(
                    psum_main[:, h * mm_n : (h + 1) * mm_n],
                    lhsT,
                    a_t[:, h * mm_n : (h + 1) * mm_n],
                    start=(jt == 0),
                    stop=(jt == jt_n - 1),
                )

        res_t = singles.tile([P, ni], f32)
        nc.scalar.copy(res_t, psum_main)

        h_ps = pp.tile([P, ni], f32, tag="hps")
        for h in range(ni // mm_n):
            hs = slice(h * mm_n, (h + 1) * mm_n)
            nc.tensor.matmul(h_ps[:, hs], bd1_t, res_t[:, hs], start=True, stop=True)
        h_t = singles.tile([P, ni], f32)
        nc.scalar.activation(
            h_t, h_ps, mybir.ActivationFunctionType.Relu, bias=bias1_t
        )

        o_ps = pp.tile([P, ni], f32, tag="ops")
        for h in range(ni // mm_n):
            hs = slice(h * mm_n, (h + 1) * mm_n)
            nc.tensor.matmul(o_ps[:, hs], bd2_t, h_t[:, hs], start=True, stop=True)
        out_t = singles.tile([P, ni], f32)
        nc.scalar.activation(
            out_t, o_ps, mybir.ActivationFunctionType.Sigmoid, bias=bias2_t
        )
        nc.sync.dma_start(out=outT_d, in_=out_t)

    nc.compile()
    return nc


def _host_prep(x, adj, time_delay, t, W1, b1, W2, b2, ncores):
    """Layout transforms only (no FLOPs of the reference are done here)."""
    x = np.ascontiguousarray(np.asarray(x, dtype=np.float32))
    adj = np.asarray(adj, dtype=np.float32)
    td = np.asarray(time_delay)
    assert td.dtype == np.int32
    b, n, f = x.shape
    ni = n // ncores
    jt_n = n // P

    thr = int(t) * 5 + 4  # time_delay // 5 <= t  <=>  time_delay <= 5t+4

    adjT = np.ascontiguousarray(adj.T)
    tdT = np.ascontiguousarray(td.T)
    # stationary x: x_sb[p, jt*P + 32*b + f] = x[b, jt*P + p, f]
    xsb = np.ascontiguousarray(
        x.reshape(b, jt_n, P, f).transpose(2, 1, 0, 3).reshape(P, jt_n * b * f)
    )
    bd1 = np.zeros((P, P), np.float32)
    bd2 = np.zeros((P, P), np.float32)
    for bb in range(b):
        bd1[bb * f : (bb + 1) * f, bb * f : (bb + 1) * f] = W1
        bd2[bb * f : (bb + 1) * f, bb * f : (bb + 1) * f] = W2
    bias1 = np.ascontiguousarray(np.tile(np.asarray(b1, np.float32), b).reshape(P, 1))
    bias2 = np.ascontiguousarray(np.tile(np.asarray(b2, np.float32), b).reshape(P, 1))

    in_maps = []
    for c in range(ncores):
        sl = slice(c * ni, (c + 1) * ni)
        in_maps.append(
            {
                "adjT": np.ascontiguousarray(adjT[:, sl]),
                "tdT": np.ascontiguousarray(tdT[:, sl]),
                "xsb": xsb,
                "bd1": bd1,
                "bd2": bd2,
                "bias1": bias1,
                "bias2": bias2,
            }
        )
    return thr, in_maps


def _run(x, adj, time_delay, t, W1, b1, W2, b2, ncores=NCORES,
         mm_dtype_name="float32", trace=False):
    from concourse.bass_utils import run_bass_kernel_spmd

    b, n, f = np.asarray(x).shape
    ni = n // ncores
    thr, in_maps = _host_prep(x, adj, time_delay, t, W1, b1, W2, b2, ncores)
    nc = _build(n, ni, thr, mm_dtype_name)
    res = run_bass_kernel_spmd(
        nc, in_maps, core_ids=list(range(ncores)), trace=trace
    )
    full = np.concatenate([r["outT"] for r in res.results], axis=1)  # [P, n]
    out = np.ascontiguousarray(full.reshape(b, f, n).transpose(0, 2, 1))
    return out, res


def kernel(x, adj, time_delay, t, W1, b1, W2, b2):
    out, _ = _run(x, adj, time_delay, t, W1, b1, W2, b2)
    return out


# revision 6
# speedup vs baseline: 1.0848x; 1.0848x over previous
"""Trainium2 Bass kernel for nn_DGNN (gnn_message_passing).

Reference computation (B=4, N=8192, F=32):
    delay_steps = time_delay // 5
    active      = (t >= delay_steps) & (adj > 0)
    A           = where(active, adj, 0)              # == adj * (time_delay <= 5*t+4)
    adjusted    = einsum('ij,bjf->bif', A, x)
    h           = relu(adjusted @ W1 + b1)
    out         = sigmoid(h @ W2 + b2)

Sharding: destination nodes i are split row-wise across 8 cores (1024 each).
Each core streams its slice of adj^T / time_delay^T ([8192 j, 1024 i], j on
partitions so the PE can contract over j), masks adj with the time-delay
predicate on the DVE, and accumulates adjusted^T in PSUM with the 4 batches
packed side-by-side in the stationary operand (partition q = 32*b + f).
The tiny per-node MLP runs on-device with block-diagonal W1/W2 so all 4
batches share one matmul. Output is returned transposed per core and
unsharded on the host.
"""

import numpy as np

B = 4
N = 8192
F = 32
P = 128
NCORES = 8
NI = N // NCORES  # dest-nodes per core
JT = N // P       # contraction tiles

MM_N = 512        # moving-operand free dim per matmul (fp32 max)


def _build(nj, ni, thr, mm_dtype_name="float32"):
    """Trace + compile the per-core Bass program.

    nj: contraction size (source nodes), ni: dest nodes per core,
    thr: mask threshold (keep edge iff time_delay <= thr).
    """
    from contextlib import ExitStack

    import concourse.bacc as bacc
    import concourse.mybir as mybir
    import concourse.tile as tile
    from concourse.dve_ops import TENSOR_MASK

    f32 = mybir.dt.float32
    i32 = mybir.dt.int32
    mm_dt = getattr(mybir.dt, mm_dtype_name)

    jt_n = nj // P
    mm_n = min(MM_N, ni)

    nc = bacc.Bacc("TRN2", target_bir_lowering=False, debug=False)

    adjT_d = nc.dram_tensor("adjT", [nj, ni], f32, kind="ExternalInput").ap()
    tdT_d = nc.dram_tensor("tdT", [nj, ni], i32, kind="ExternalInput").ap()
    xsb_d = nc.dram_tensor("xsb", [P, jt_n * P], f32, kind="ExternalInput").ap()
    bd1_d = nc.dram_tensor("bd1", [P, P], f32, kind="ExternalInput").ap()
    bd2_d = nc.dram_tensor("bd2", [P, P], f32, kind="ExternalInput").ap()
    bias1_d = nc.dram_tensor("bias1", [P, 1], f32, kind="ExternalInput").ap()
    bias2_d = nc.dram_tensor("bias2", [P, 1], f32, kind="ExternalInput").ap()
    outT_d = nc.dram_tensor("outT", [P, ni], f32, kind="ExternalOutput").ap()

    # x is preloaded in chunks interleaved with the main stream so the big
    # stationary tensor doesn't delay pipeline start.
    x_chunks = max(1, jt_n // 8)
    jt_per_chunk = jt_n // x_chunks

    with tile.TileContext(nc) as tc, ExitStack() as ctx:
        io = ctx.enter_context(tc.tile_pool(name="io", bufs=5))
        wrk = ctx.enter_context(tc.tile_pool(name="wrk", bufs=3))
        singles = ctx.enter_context(tc.tile_pool(name="singles", bufs=1))
        pp = ctx.enter_context(tc.tile_pool(name="pp", bufs=1, space="PSUM"))

        bd1_t = singles.tile([P, P], f32)
        nc.scalar.dma_start(out=bd1_t, in_=bd1_d)
        bd2_t = singles.tile([P, P], f32)
        nc.scalar.dma_start(out=bd2_t, in_=bd2_d)
        bias1_t = singles.tile([P, 1], f32)
        nc.scalar.dma_start(out=bias1_t, in_=bias1_d)
        bias2_t = singles.tile([P, 1], f32)
        nc.scalar.dma_start(out=bias2_t, in_=bias2_d)

        # Pre-warm the ACT function tables (Relu+Sigmoid) so the table DMAs
        # overlap the main stream instead of landing in the kernel tail.
        warm_t = singles.tile([P, 1], f32)
        nc.scalar.activation(
            warm_t, bias1_t, mybir.ActivationFunctionType.Relu, bias=bias1_t
        )
        nc.scalar.activation(
            warm_t, bias1_t, mybir.ActivationFunctionType.Sigmoid, bias=bias1_t
        )

        x_t = singles.tile([P, jt_n * P], f32)
        psum_main = pp.tile([P, ni], f32)

        for jt in range(jt_n):
            if jt % jt_per_chunk == 0:
                c = jt // jt_per_chunk
                cs = slice(c * jt_per_chunk * P, (c + 1) * jt_per_chunk * P)
                nc.scalar.dma_start(out=x_t[:, cs], in_=xsb_d[:, cs])
            td_t = io.tile([P, ni], i32, tag="td")
            nc.sync.dma_start(out=td_t, in_=tdT_d[jt * P : (jt + 1) * P, :])
            adj_t = io.tile([P, ni], f32, tag="adj")
            nc.scalar.dma_start(out=adj_t, in_=adjT_d[jt * P : (jt + 1) * P, :])

            # A = where(time_delay <= thr, adj, 0) in one fused DVE op:
            # TENSOR_MASK: out[k] = select(in1[k] + c2 < c0, in0[k], 0)
            a_t = wrk.tile([P, ni], mm_dt, tag="a")
            nc.vector._custom_dve(
                TENSOR_MASK, out=a_t, in0=adj_t, in1=td_t,
                s0=float(thr) + 0.5, s1=0.0, imm2=0.0,
            )

            lhsT = x_t[:, jt * P : (jt + 1) * P]
            if mm_dt != f32:
                lhsT = lhsT.bitcast(mm_dt)
            for h in range(ni // mm_n):
                nc.tensor.matmul(
                    psum_main[:, h * mm_n : (h + 1) * mm_n],
                    lhsT,
                    a_t[:, h * mm_n : (h + 1) * mm_n],
                    start=(jt == 0),
                    stop=(jt == jt_n - 1),
                )

        # Per-node MLP, pipelined in independent column halves.
        h_ps = pp.tile([P, ni], f32, tag="hps")
        o_ps = pp.tile([P, ni], f32, tag="ops")
        for h in range(ni // mm_n):
            hs = slice(h * mm_n, (h + 1) * mm_n)
            res_t = singles.tile([P, mm_n], f32, tag=f"res{h}", name=f"res{h}")
            nc.scalar.copy(res_t, psum_main[:, hs])
            nc.tensor.matmul(h_ps[:, hs], bd1_t, res_t, start=True, stop=True)
            h_t = singles.tile([P, mm_n], f32, tag=f"h{h}", name=f"h{h}")
            nc.scalar.activation(
                h_t, h_ps[:, hs], mybir.ActivationFunctionType.Relu, bias=bias1_t
            )
            nc.tensor.matmul(o_ps[:, hs], bd2_t, h_t, start=True, stop=True)
            out_t = singles.tile([P, mm_n], f32, tag=f"out{h}", name=f"out{h}")
            nc.scalar.activation(
                out_t, o_ps[:, hs], mybir.ActivationFunctionType.Sigmoid, bias=bias2_t
            )
            nc.sync.dma_start(out=outT_d[:, hs], in_=out_t)

    nc.compile()
    return nc


def _host_prep(x, adj, time_delay, t, W1, b1, W2, b2, ncores):
    """Layout transforms only (no FLOPs of the reference are done here)."""
    x = np.ascontiguousarray(np.asarray(x, dtype=np.float32))
    adj = np.asarray(adj, dtype=np.float32)
    td = np.asarray(time_delay)
    assert td.dtype == np.int32
    b, n, f = x.shape
    ni = n // ncores
    jt_n = n // P

    thr = int(t) * 5 + 4  # time_delay // 5 <= t  <=>  time_delay <= 5t+4

    adjT = np.ascontiguousarray(adj.T)
    tdT = np.ascontiguousarray(td.T)
    # stationary x: x_sb[p, jt*P + 32*b + f] = x[b, jt*P + p, f]
    xsb = np.ascontiguousarray(
        x.reshape(b, jt_n, P, f).transpose(2, 1, 0, 3).reshape(P, jt_n * b * f)
    )
    bd1 = np.zeros((P, P), np.float32)
    bd2 = np.zeros((P, P), np.float32)
    for bb in range(b):
        bd1[bb * f : (bb + 1) * f, bb * f : (bb + 1) * f] = W1
        bd2[bb * f : (bb + 1) * f, bb * f : (bb + 1) * f] = W2
    bias1 = np.ascontiguousarray(np.tile(np.asarray(b1, np.float32), b).reshape(P, 1))
    bias2 = np.ascontiguousarray(np.tile(np.asarray(b2, np.float32), b).reshape(P, 1))

    in_maps = []
    for c in range(ncores):
        sl = slice(c * ni, (c + 1) * ni)
        in_maps.append(
            {
                "adjT": np.ascontiguousarray(adjT[:, sl]),
                "tdT": np.ascontiguousarray(tdT[:, sl]),
                "xsb": xsb,
                "bd1": bd1,
                "bd2": bd2,
                "bias1": bias1,
                "bias2": bias2,
            }
        )
    return thr, in_maps


def _run(x, adj, time_delay, t, W1, b1, W2, b2, ncores=NCORES,
         mm_dtype_name="float32", trace=False):
    from concourse.bass_utils import run_bass_kernel_spmd

    b, n, f = np.asarray(x).shape
    ni = n // ncores
    thr, in_maps = _host_prep(x, adj, time_delay, t, W1, b1, W2, b2, ncores)
    nc = _build(n, ni, thr, mm_dtype_name)
    res = run_bass_kernel_spmd(
        nc, in_maps, core_ids=list(range(ncores)), trace=trace
    )
    full = np.concatenate([r["outT"] for r in res.results], axis=1)  # [P, n]
    out = np.ascontiguousarray(full.reshape(b, f, n).transpose(0, 2, 1))
    return out, res


def kernel(x, adj, time_delay, t, W1, b1, W2, b2):
    out, _ = _run(x, adj, time_delay, t, W1, b1, W2, b2)
    return out
